# revision 24
# baseline (speedup 1.0000x reference)
"""Trainium2 Bass kernel for nn_CNN_90546500534707 (segment_reduce).

Pipeline (per the reference):
  1. per-mask channel means over masked pixels, sequential overwrite ->
     fsm = x - mean[last valid mask covering pixel]  (output 1)
  2. conv stack 256->128->64->32 (3x3, SAME, relu on first two)
  3. per-mask covariance of conv features (32x32) -> fc 1024->1024 (output 0)

Sharding: image rows split across 8 cores (32 rows + 3-row halo each).
Everything is laid out in a zero-padded row geometry (width 258) so 3x3
convs become 9 shifted matmuls accumulated in PSUM.  The only cross-core
exchange is one AllReduce of the per-mask channel sums [9, 256].  The
per-mask covariance contribution and the fc output are linear in the
per-core partial covariance, so each core emits a partial `trans` and the
host sums them (and adds fc bias / validity masking).

Host-side prep (mask munging, padding, weight transposes) is O(masks+
weights) bookkeeping; all pixel-volume compute runs on the NeuronCores.
"""

import sys

sys.path.insert(0, "/opt/trn_rl_repo")

import numpy as np

import concourse.bass as bass
import concourse.tile as tile
from concourse import bacc, mybir
from concourse.bass_utils import run_bass_kernel_spmd
from concourse.tile_rust import add_dep_helper


def _chain(insts):
    """Order matmuls of one PSUM bank group (start must execute first)."""
    for a, b in zip(insts[1:], insts):
        add_dep_helper(a.ins, b.ins, sync=False, reason="psum group order")

F32 = mybir.dt.float32
F32R = mybir.dt.float32r
BF16 = mybir.dt.bfloat16
AF = mybir.ActivationFunctionType
ALU = mybir.AluOpType


class Cfg:
    def __init__(self, C=256, H=256, W=256, K=9, OC1=128, OC2=64, OC3=32,
                 n_cores=8):
        assert C % 128 == 0
        self.C, self.H, self.W, self.K = C, H, W, K
        self.OC1, self.OC2, self.OC3 = OC1, OC2, OC3
        self.NCHUNK = C // 128
        self.n_cores = n_cores
        self.R = H // n_cores            # own rows per core
        self.Wp = W + 2                  # padded row width
        self.RX = self.R + 8             # x/fsm buffer rows (3 halo + 1 pad per side)
        self.PXB = self.RX * self.Wp
        self.RC1 = self.R + 6            # conv1 out rows (R+4) + 2 pad rows
        self.PC1 = self.RC1 * self.Wp
        self.RC2 = self.R + 4            # conv2 out rows (R+2) + 2 pad rows
        self.PC2 = self.RC2 * self.Wp
        self.P3 = self.R * self.Wp       # conv3/cov pixel region
        self.NB = -(-(self.R * self.Wp) // 128)     # 128-px blocks for S/cov
        self.NW3 = -(-self.P3 // 512)    # conv3 windows
        self.NP3 = -(-self.NW3 // 2)     # conv3 pair groups (f32r col-tiling x2)
        self.FCN = self.OC3 * self.OC3   # 1024
        self.FCCH = -(-self.FCN // 128)  # fc contraction chunks (8)
        # sanity for buffer over-reads (garbage reads stay in-bounds)
        assert 4 * self.Wp + 128 * self.NB <= self.PXB
        assert self.P3 + 3 * self.Wp + 2 <= self.PC2 + 9
        assert 128 * self.NB <= self.NW3 * 512


def build(cfg: Cfg):
    """Builds the SPMD program; returns compiled Bacc."""
    nc = bacc.Bacc("TRN2", target_bir_lowering=False, debug=False,
                   num_devices=cfg.n_cores)
    C, K, Wp, R = cfg.C, cfg.K, cfg.Wp, cfg.R
    NCH = cfg.NCHUNK
    OC1, OC2, OC3 = cfg.OC1, cfg.OC2, cfg.OC3
    TAPS = [(dy, dx) for dy in (-1, 0, 1) for dx in (-1, 0, 1)]

    # ---------------- I/O ----------------
    x_in = nc.dram_tensor("x_slab", [NCH, 128, cfg.PXB], F32R, kind="ExternalInput")
    sel_in = nc.dram_tensor("sel", [K, cfg.PXB], BF16, kind="ExternalInput")
    fgT_in = nc.dram_tensor("fgT", [128, cfg.NB, K], F32R, kind="ExternalInput")
    rc9_in = nc.dram_tensor("rc9", [K, 1], F32, kind="ExternalInput")
    w1_in = nc.dram_tensor("w1T", [9, NCH, 128, OC1], F32R, kind="ExternalInput")
    w2_in = nc.dram_tensor("w2T", [9, OC1, OC2], BF16, kind="ExternalInput")
    w3_in = nc.dram_tensor("w3T", [9, OC2, OC3], BF16, kind="ExternalInput")
    b1_in = nc.dram_tensor("b1c", [OC1, 1], F32, kind="ExternalInput")
    b2_in = nc.dram_tensor("b2c", [OC2, 1], F32, kind="ExternalInput")
    b3_in = nc.dram_tensor("b3c", [4 * OC3, 1], F32, kind="ExternalInput")
    fcw_in = nc.dram_tensor("fcwT", [cfg.FCCH, 128, cfg.FCN], F32R, kind="ExternalInput")
    id128_in = nc.dram_tensor("iden128", [128, 128], F32R, kind="ExternalInput")
    id32_in = nc.dram_tensor("iden32", [128, OC3], F32R, kind="ExternalInput")
    mc1_in = nc.dram_tensor("mc1", [4, 1], F32, kind="ExternalInput")
    mc2_in = nc.dram_tensor("mc2", [2, 1], F32, kind="ExternalInput")

    fsm_out = nc.dram_tensor("fsm_o", [NCH, 128, R, cfg.W], F32, kind="ExternalOutput")
    tr_out = nc.dram_tensor("tr_o", [K, cfg.FCN], F32, kind="ExternalOutput")

    with tile.TileContext(nc) as tc:
        with (
            tc.tile_pool(name="big", bufs=1) as big,
            tc.tile_pool(name="wts", bufs=1) as wts,
            tc.tile_pool(name="small", bufs=1) as small,
            tc.tile_pool(name="selp", bufs=2) as selp,
            tc.tile_pool(name="xt", bufs=2) as xtp,
            tc.tile_pool(name="fvt", bufs=3) as fvtp,
            tc.tile_pool(name="mst", bufs=2) as mstp,
            tc.tile_pool(name="fcw", bufs=1) as fcwp,
            tc.tile_pool(name="ps", bufs=4, space="PSUM") as psp,
            tc.tile_pool(name="dram", bufs=1, space="DRAM") as dram,
        ):
            # ------------- load big/static data -------------
            xb = [big.tile([128, cfg.PXB], F32R, name=f"xb{q}") for q in range(NCH)]
            nrow4 = -(-cfg.RX // 4)
            for p in range(4):
                lo = p * nrow4 * Wp
                hi = min(cfg.PXB, (p + 1) * nrow4 * Wp)
                for q in range(NCH):
                    nc.sync.dma_start(xb[q][:, lo:hi], x_in.ap()[q][:, lo:hi])

            c1b = big.tile([128, cfg.PC1 + 8], BF16, name="c1b")
            c2b = big.tile([OC2, cfg.PC2 + 9], BF16, name="c2b")  # +1 leading guard elem
            fvb = big.tile([128, cfg.NP3 * 512], F32R, name="fvb")
            nc.vector.memset(c1b[:, :Wp], 0.0)                       # top pad row
            nc.vector.memset(c1b[:, (cfg.RC1 - 1) * Wp:], 0.0)       # bottom pad + tail
            nc.vector.memset(c2b[:, :1 + Wp], 0.0)
            nc.vector.memset(c2b[:, 1 + (cfg.RC2 - 1) * Wp:], 0.0)
            nc.vector.memset(fvb[:, :].bitcast(F32), 0.0)

            w1s = wts.tile([128, 9 * NCH, OC1], F32R, name="w1s")
            nc.sync.dma_start(w1s[:], w1_in.ap().rearrange("t q i o -> i (t q) o"))
            w2s = wts.tile([OC1, 9, OC2], BF16, name="w2s")
            nc.sync.dma_start(w2s[:], w2_in.ap().rearrange("t i o -> i t o"))
            w3s = wts.tile([OC2, 9, OC3], BF16, name="w3s")
            nc.sync.dma_start(w3s[:], w3_in.ap().rearrange("t i o -> i t o"))
            fgTs = wts.tile([128, cfg.NB, K], F32R, name="fgTs")
            nc.sync.dma_start(fgTs[:], fgT_in.ap())
            rc9 = small.tile([K, 1], F32, name="rc9")
            nc.sync.dma_start(rc9[:], rc9_in.ap())

            b1s = small.tile([OC1, 1], F32, name="b1s")
            nc.sync.dma_start(b1s[:], b1_in.ap())
            b2s = small.tile([OC2, 1], F32, name="b2s")
            nc.sync.dma_start(b2s[:], b2_in.ap())
            b3s = small.tile([4 * OC3, 1], F32, name="b3s")
            nc.sync.dma_start(b3s[:], b3_in.ap())
            id128 = small.tile([128, 128], F32R, name="id128")
            nc.sync.dma_start(id128[:], id128_in.ap())
            id32 = small.tile([128, OC3], F32R, name="id32")
            nc.sync.dma_start(id32[:], id32_in.ap())
            mc1s = small.tile([128, 4], F32, name="mc1s")
            _a = mc1_in.ap()
            nc.sync.dma_start(mc1s[:], bass.AP(
                tensor=_a.tensor, offset=0, ap=[[0, 128]] + list(_a.ap)))
            mc2s = small.tile([OC2, 2], F32, name="mc2s")
            _a = mc2_in.ap()
            nc.sync.dma_start(mc2s[:], bass.AP(
                tensor=_a.tensor, offset=0, ap=[[0, OC2]] + list(_a.ap)))

            # ------------- phase A: masked channel sums S -> means -------------
            # S[i, c] = sum_px fgT[px, i] * x[c, px]   (fgT host-scaled by 1/cnt)
            s_acc = psp.tile([K, C], F32, name="s_acc", tag="acc", bufs=2)
            BB = 512 // (128 * NCH)          # S-blocks per PSUM bank
            for jj in range(0, cfg.NB, BB):
                blks = range(jj, min(jj + BB, cfg.NB))
                xt_ps = psp.tile([128, 512], F32R, name="xt_ps", tag="tps", bufs=2)
                tidx = 0
                ntr = len(blks) * NCH
                tr_insts = []
                for j in blks:
                    off = 4 * Wp + 128 * j
                    for q in range(NCH):
                        tr_insts.append(nc.tensor.matmul(
                            xt_ps[:, 128 * (NCH * (j - jj) + q):
                                  128 * (NCH * (j - jj) + q + 1)],
                            xb[q][:, off:off + 128], id128[:],
                            is_transpose=True,
                            start=(tidx == 0), stop=(tidx == ntr - 1)))
                        tidx += 1
                _chain(tr_insts)
                for j in blks:
                    xt_sb = xtp.tile([128, 128 * NCH], F32R, name="xt_sb")
                    nc.vector.tensor_copy(
                        xt_sb[:],
                        xt_ps[:, 128 * NCH * (j - jj):128 * NCH * (j - jj + 1)])
                    nc.tensor.matmul(
                        s_acc[:], fgTs[:, j, :], xt_sb[:],
                        start=(j == 0), stop=(j == cfg.NB - 1))

            s_sb = small.tile([K, C], F32, name="s_sb")
            nc.vector.tensor_copy(s_sb[:], s_acc[:])

            # AllReduce S across cores -> means
            ar_in = dram.tile([K, C], F32, name="ar_in")
            ar_out = dram.tile([K, C], F32, name="ar_out", addr_space="Shared")
            nc.gpsimd.dma_start(ar_in[:], s_sb[:])
            nc.gpsimd.collective_compute(
                "AllReduce", ALU.add,
                replica_groups=[list(range(cfg.n_cores))],
                ins=[ar_in.opt()], outs=[ar_out.opt()])
            means = small.tile([K, C], F32, name="means")
            nc.gpsimd.dma_start(means[:], ar_out[:])
            nc.vector.tensor_scalar_mul(means[:], means[:], rc9[:, 0:1])
            means_r = small.tile([K, C], BF16, name="means_r")
            nc.scalar.copy(means_r[:], means[:])

            # ------------- phase B: fsm = x - mean[sel] (in place on xb) ------
            fs_lo = Wp
            fs_hi = (cfg.RX - 1) * Wp
            w = fs_lo
            while w < fs_hi:
                nw = min(512, fs_hi - w)
                selw = selp.tile([K, 512], BF16, name="selw")
                nc.sync.dma_start(selw[:, :nw], sel_in.ap()[:, w:w + nw])
                for q in range(NCH):
                    msel = psp.tile([128, 512], F32, name="msel", tag="cps")
                    nc.tensor.matmul(
                        msel[:, :nw], means_r[:, 128 * q:128 * (q + 1)],
                        selw[:, :nw], start=True, stop=True)
                    nc.vector.tensor_tensor(
                        xb[q][:, w:w + nw], xb[q][:, w:w + nw],
                        msel[:, :nw], ALU.subtract)
                w += nw

            # write fsm output (own rows, real cols)
            for q in range(NCH):
                nc.sync.dma_start(
                    fsm_out.ap()[q],
                    xb[q][:, :].bitcast(F32).rearrange(
                        "p (r u) -> p r u", r=cfg.RX)[:, 4:4 + R, 1:1 + cfg.W])

            # ------------- phase C: conv1 (C -> OC1, relu) -------------
            c1_lo, c1_hi = Wp, (cfg.RC1 - 1) * Wp
            wins = []
            w = c1_lo
            while w < c1_hi:
                wins.append((w, min(512, c1_hi - w)))
                w += 512
            for g in range(0, len(wins), 4):
                grp = wins[g:g + 4]
                psl = [psp.tile([128, 512], F32, name="c1ps", tag="cps") for _ in grp]
                for ti, (dy, dx) in enumerate(TAPS):
                    for q in range(NCH):
                        tq = ti * NCH + q
                        for (s, nw), ps in zip(grp, psl):
                            nc.tensor.matmul(
                                ps[:OC1, :nw], w1s[:, tq, :],
                                xb[q][:, s + Wp + dy * Wp + dx:
                                      s + Wp + dy * Wp + dx + nw],
                                start=(tq == 0), stop=(tq == 9 * NCH - 1))
                for (s, nw), ps in zip(grp, psl):
                    nc.scalar.activation(c1b[:OC1, s:s + nw], ps[:OC1, :nw],
                                         AF.Relu, bias=b1s[:, 0:1])
            # re-zero pad columns (conv2 padding)
            c1v = c1b[:, :cfg.PC1].rearrange("p (r u) -> p r u", r=cfg.RC1)
            nc.vector.memset(c1v[:, 1:cfg.RC1 - 1, 0:1], 0.0)
            nc.vector.memset(c1v[:, 1:cfg.RC1 - 1, Wp - 1:Wp], 0.0)
            # zero conv1 rows outside the image (conv2 expects zero padding)
            c1m = c1b[:, Wp:3 * Wp].rearrange("p (a b) -> p a b", a=2)
            nc.vector.tensor_tensor(
                c1m, c1m, mc1s[:, 0:2].unsqueeze(2).broadcast_to(
                    [128, 2, Wp]), ALU.mult)
            c1m = c1b[:, (cfg.RC1 - 3) * Wp:(cfg.RC1 - 1) * Wp].rearrange(
                "p (a b) -> p a b", a=2)
            nc.vector.tensor_tensor(
                c1m, c1m, mc1s[:, 2:4].unsqueeze(2).broadcast_to(
                    [128, 2, Wp]), ALU.mult)

            # ------------- phase D: conv2 (OC1 -> OC2, relu) -------------
            c2_lo, c2_hi = Wp, (cfg.RC2 - 1) * Wp
            wins = []
            w = c2_lo
            while w < c2_hi:
                wins.append((w, min(512, c2_hi - w)))
                w += 512
            for g in range(0, len(wins), 4):
                grp = wins[g:g + 4]
                psl = [psp.tile([128, 512], F32, name="c2ps", tag="cps") for _ in grp]
                for ti, (dy, dx) in enumerate(TAPS):
                    for (s, nw), ps in zip(grp, psl):
                        nc.tensor.matmul(
                            ps[:OC2, :nw], w2s[:, ti, :],
                            c1b[:, s + Wp + dy * Wp + dx:
                                s + Wp + dy * Wp + dx + nw],
                            start=(ti == 0), stop=(ti == 8))
                for (s, nw), ps in zip(grp, psl):
                    nc.scalar.activation(c2b[:, 1 + s:1 + s + nw], ps[:OC2, :nw],
                                         AF.Relu, bias=b2s[:, 0:1])
            c2v = c2b[:, 1:1 + cfg.PC2].rearrange("p (r u) -> p r u", r=cfg.RC2)
            nc.vector.memset(c2v[:, 1:cfg.RC2 - 1, 0:1], 0.0)
            nc.vector.memset(c2v[:, 1:cfg.RC2 - 1, Wp - 1:Wp], 0.0)
            # zero conv2 rows outside the image (conv3 expects zero padding)
            nc.vector.tensor_tensor(
                c2b[:, 1 + Wp:1 + 2 * Wp], c2b[:, 1 + Wp:1 + 2 * Wp],
                mc2s[:, 0:1].broadcast_to([OC2, Wp]), ALU.mult)
            nc.vector.tensor_tensor(
                c2b[:, 1 + (cfg.RC2 - 2) * Wp:1 + (cfg.RC2 - 1) * Wp],
                c2b[:, 1 + (cfg.RC2 - 2) * Wp:1 + (cfg.RC2 - 1) * Wp],
                mc2s[:, 1:2].broadcast_to([OC2, Wp]), ALU.mult)

            # ------------- phase E: conv3 (OC2 -> OC3, +bias), col-tiled x4 ----
            for qd in range(cfg.NP3):
                wlist = [wi for wi in range(2 * qd, min(2 * qd + 2, cfg.NW3))]
                ps = psp.tile([128, 512], F32, name="c3ps", tag="cps")
                for ti, (dy, dx) in enumerate(TAPS):
                    for wi in wlist:
                        s = 512 * wi
                        nw = min(512, cfg.P3 - s)
                        b = wi % 2
                        nc.tensor.matmul(
                            ps[64 * b:64 * b + OC3, :nw],
                            w3s[:, ti, :],
                            c2b[:, 1 + s + 2 * Wp + dy * Wp + dx:
                                1 + s + 2 * Wp + dy * Wp + dx + nw],
                            start=(ti == 0), stop=(ti == 8),
                            tile_position=(0, 64 * b),
                            skip_group_check=True)
                for wi in wlist:
                    nw = min(512, cfg.P3 - 512 * wi)
                    b = wi % 2
                    nc.scalar.activation(
                        fvb[64 * b:64 * b + OC3, 512 * qd:512 * qd + nw],
                        ps[64 * b:64 * b + OC3, :nw],
                        AF.Identity, bias=b3s[64 * b:64 * b + OC3, 0:1])

            # ------------- phase F: per-mask covariance -------------
            cov_ps = psp.tile([OC3, K * OC3], F32, name="cov_ps", tag="acc",
                              bufs=2)
            for wi in range(cfg.NW3):
                blks = [j for j in range(4 * wi, min(4 * wi + 4, cfg.NB))]
                if not blks:
                    break
                b2 = 64 * (wi % 2)
                k4 = wi // 2
                fvt_ps = psp.tile([128, 4 * OC3], F32R, name="fvt_ps",
                                  tag="tps", bufs=2)
                tr_insts = []
                for j in blks:
                    woff = 128 * (j - 4 * wi)
                    tr_insts.append(nc.tensor.matmul(
                        fvt_ps[:, OC3 * (j - 4 * wi):OC3 * (j - 4 * wi + 1)],
                        fvb[b2:b2 + OC3, 512 * k4 + woff:
                            512 * k4 + woff + 128],
                        id32[b2:b2 + OC3, :],
                        is_transpose=True,
                        start=(j == blks[0]), stop=(j == blks[-1]),
                        tile_position=(b2, 0)))
                _chain(tr_insts)
                nb = OC3 * len(blks)
                fvt = fvtp.tile([128, 4 * OC3], F32R, name="fvt")
                nc.vector.tensor_copy(fvt[:, :nb], fvt_ps[:, :nb])
                for j in blks:
                    jo = OC3 * (j - 4 * wi)
                    mst = mstp.tile([128, K, OC3], F32R, name="mst")
                    nc.vector.tensor_tensor(
                        mst[:],
                        fvt[:, jo:jo + OC3].unsqueeze(1).broadcast_to([128, K, OC3]),
                        fgTs[:, j, :].unsqueeze(2).broadcast_to([128, K, OC3]),
                        ALU.mult)
                    nc.tensor.matmul(
                        cov_ps[:], fvt[:, jo:jo + OC3],
                        mst[:, :, :].rearrange("p k d -> p (k d)"),
                        start=(j == 0), stop=(j == cfg.NB - 1))

            cov_sb = small.tile([OC3, K * OC3], F32R, name="cov_sb")
            nc.vector.tensor_copy(cov_sb[:], cov_ps[:])

            # rearrange cov[c,(k,d)] -> covT[(c4,d), (ci,k)] via DRAM bounce
            ckcd = dram.tile([K, OC3, OC3], F32R, name="ckcd")
            nc.gpsimd.dma_start(
                ckcd.rearrange("k c d -> c k d"),
                cov_sb.rearrange("c (k d) -> c k d", k=K))
            covTs = small.tile([128, cfg.FCCH, K], F32R, name="covTs")
            for ci in range(cfg.FCCH):
                nc.gpsimd.dma_start(
                    covTs[:, ci, :],
                    ckcd[:, 4 * ci:4 * ci + 4, :].rearrange(
                        "k c d -> (c d) k"))

            # ------------- phase G: fc (partial trans) -------------
            NJ = cfg.FCN
            halves = [(h, min(512, NJ - h)) for h in range(0, NJ, 512)]
            tr_ps = [psp.tile([K, min(512, NJ)], F32, name=f"tr_ps{i}",
                              tag="acc", bufs=2) for i in range(len(halves))]
            for ci in range(cfg.FCCH):
                for (h, nh), ps in zip(halves, tr_ps):
                    fcw = fcwp.tile([128, 512], F32R, name="fcw")
                    nc.sync.dma_start(fcw[:, :nh], fcw_in.ap()[ci, :, h:h + nh])
                    nc.tensor.matmul(
                        ps[:, :nh], covTs[:, ci, :], fcw[:, :nh],
                        start=(ci == 0), stop=(ci == cfg.FCCH - 1))
            tr_sb = small.tile([K, NJ], F32, name="tr_sb")
            for (h, nh), ps in zip(halves, tr_ps):
                nc.vector.tensor_copy(tr_sb[:, h:h + nh], ps[:, :nh])
            nc.sync.dma_start(tr_out.ap()[:], tr_sb[:])

    nc.compile()
    return nc


# ============================ host side ============================

def prep_inputs(cfg: Cfg, x, masks, w1, b1, w2, b2, w3, b3, fc_w, fc_b):
    C, H, W, K, Wp, R = cfg.C, cfg.H, cfg.W, cfg.K, cfg.Wp, cfg.R
    xv = np.asarray(x, np.float32).reshape(C, H, W)
    m = np.asarray(masks)
    fg = (m > 0).astype(np.float32)                      # [K, H, W]
    counts = fg.reshape(K, -1).sum(1)
    valid = counts >= 10.0
    recip = (1.0 / np.maximum(counts, 1.0)).astype(np.float32)

    # one-hot of the last valid mask covering each pixel
    sel = np.zeros((K, H, W), np.float32)
    covered = np.zeros((H, W), bool)
    for i in range(K - 1, -1, -1):
        if not valid[i]:
            continue
        on = fg[i] > 0
        sel[i][on & ~covered] = 1.0
        covered |= on


    import ml_dtypes
    w1 = np.asarray(w1, np.float32)
    w2 = np.asarray(w2, np.float32)
    w3 = np.asarray(w3, np.float32)
    w1T = np.ascontiguousarray(np.transpose(
        w1.reshape(cfg.OC1, cfg.NCHUNK, 128, 3, 3),
        (3, 4, 1, 2, 0)).reshape(9, cfg.NCHUNK, 128, cfg.OC1))
    w2T = np.ascontiguousarray(np.transpose(w2, (2, 3, 1, 0)).reshape(
        9, cfg.OC1, cfg.OC2)).astype(ml_dtypes.bfloat16)
    w3T = np.ascontiguousarray(np.transpose(w3, (2, 3, 1, 0)).reshape(
        9, cfg.OC2, cfg.OC3)).astype(ml_dtypes.bfloat16)
    fcwT = np.ascontiguousarray(
        np.asarray(fc_w, np.float32).T.reshape(cfg.FCCH, 128, cfg.FCN))
    id128 = np.eye(128, dtype=np.float32)
    id32 = np.tile(np.eye(cfg.OC3, dtype=np.float32), (128 // cfg.OC3, 1))
    b1c = np.asarray(b1, np.float32).reshape(-1, 1)
    b2c = np.asarray(b2, np.float32).reshape(-1, 1)
    b3c = np.tile(np.asarray(b3, np.float32), 4).reshape(-1, 1)

    in_maps = []
    for c in range(cfg.n_cores):
        r0 = c * R
        xs = np.zeros((cfg.NCHUNK, 128, cfg.RX, Wp), np.float32)
        ss = np.zeros((K, cfg.RX, Wp), np.float32)
        for b in range(1, cfg.RX - 1):
            r = r0 - 4 + b
            if 0 <= r < H:
                xs[:, :, b, 1:1 + W] = xv.reshape(cfg.NCHUNK, 128, H, W)[:, :, r, :]
                ss[:, b, 1:1 + W] = sel[:, r, :]
        fgw = np.zeros((cfg.RX, Wp, K), np.float32)
        fgw[4:4 + R, 1:1 + W, :] = np.transpose(fg[:, r0:r0 + R, :], (1, 2, 0))
        fgflat = fgw.reshape(-1, K)
        base = 4 * Wp
        fgT = np.transpose(
            fgflat[base:base + cfg.NB * 128].reshape(cfg.NB, 128, K),
            (1, 0, 2)).astype(np.float32)
        mc1 = np.array([[1.0 if 0 <= r < H else 0.0]
                        for r in (r0 - 2, r0 - 1, r0 + R, r0 + R + 1)],
                       np.float32)
        mc2 = np.array([[1.0 if 0 <= r < H else 0.0]
                        for r in (r0 - 1, r0 + R)], np.float32)
        in_maps.append({
            "mc1": mc1, "mc2": mc2,
            "x_slab": xs.reshape(cfg.NCHUNK, 128, cfg.PXB),
            "sel": ss.reshape(K, cfg.PXB).astype(ml_dtypes.bfloat16),
            "fgT": np.ascontiguousarray(fgT),
            "rc9": recip.reshape(K, 1),
            "w1T": w1T, "w2T": w2T, "w3T": w3T,
            "b1c": b1c, "b2c": b2c, "b3c": b3c,
            "fcwT": fcwT, "iden128": id128, "iden32": id32,
        })
    return in_maps, valid, counts


def assemble(cfg: Cfg, results, valid, fc_b, recip=None):
    K = cfg.K
    fsm = np.zeros((cfg.C, cfg.H, cfg.W), np.float32)
    for c, res in enumerate(results):
        r0 = c * cfg.R
        fo = res["fsm_o"]           # [NCH, 128, R, W]
        for q in range(cfg.NCHUNK):
            fsm[q * 128:(q + 1) * 128, r0:r0 + cfg.R, :] = fo[q]
    trans = np.zeros((K, cfg.FCN), np.float32)
    for res in results:
        trans += res["tr_o"]
    if recip is not None:
        trans *= recip[:, None]
    trans = trans + np.asarray(fc_b, np.float32)[None, :]
    trans[~valid] = 0.0
    return trans.astype(np.float32), fsm.reshape(cfg.C, cfg.H * cfg.W)


_CACHE = {}


def _get_nc(cfg: Cfg):
    key = (cfg.C, cfg.H, cfg.W, cfg.n_cores)
    if key not in _CACHE:
        _CACHE[key] = build(cfg)
    return _CACHE[key]


def kernel(x, masks, w1, b1, w2, b2, w3, b3, fc_w, fc_b, **run_kwargs):
    cfg = Cfg(C=x.shape[1], H=x.shape[2], W=x.shape[3])
    nc = _get_nc(cfg)
    in_maps, valid, counts = prep_inputs(cfg, x, masks, w1, b1, w2, b2, w3,
                                         b3, fc_w, fc_b)
    recip = (1.0 / np.maximum(counts, 1.0)).astype(np.float32)
    res = run_bass_kernel_spmd(nc, in_maps, core_ids=list(range(cfg.n_cores)),
                               **run_kwargs)
    out = assemble(cfg, res.results, valid, fc_b, recip)
    kernel.last_results = res
    return out


# revision 27
# speedup vs baseline: 1.1401x; 1.1401x over previous
"""Trainium2 Bass kernel for nn_CNN_90546500534707 (segment_reduce).

Pipeline (per the reference):
  1. per-mask channel means over masked pixels, sequential overwrite ->
     fsm = x - mean[last valid mask covering pixel]  (output 1)
  2. conv stack 256->128->64->32 (3x3, SAME, relu on first two)
  3. per-mask covariance of conv features (32x32) -> fc 1024->1024 (output 0)

Sharding: image rows split across 8 cores (32 rows + 3-row halo each).
Everything is laid out in a zero-padded row geometry (width W+2) so 3x3
convs become 9 shifted matmuls accumulated in PSUM.  The only cross-core
exchange is one AllReduce of the per-mask channel sums [9, 256]; the
per-mask covariance and fc are linear in the per-core partial covariance,
so each core emits a partial `trans` and the host sums/scales them.

Precision: x, fsm, means and the masked-sum path stay fp32/near-exact; the
conv stack runs bf16 (fp32 PSUM accumulation); covariance + fc run f32r.
"""

import sys

sys.path.insert(0, "/opt/trn_rl_repo")

import numpy as np
import ml_dtypes

import concourse.bass as bass
import concourse.tile as tile
from concourse import bacc, mybir
from concourse.bass_utils import run_bass_kernel_spmd
from concourse.tile_rust import add_dep_helper

F32 = mybir.dt.float32
F32R = mybir.dt.float32r
BF16 = mybir.dt.bfloat16
AF = mybir.ActivationFunctionType
ALU = mybir.AluOpType
BFNP = ml_dtypes.bfloat16


def _chain(insts):
    """Order matmuls of one PSUM bank group (start must execute first)."""
    for a, b in zip(insts[1:], insts):
        add_dep_helper(a.ins, b.ins, sync=False, reason="psum group order")


class Cfg:
    def __init__(self, C=256, H=256, W=256, K=9, OC1=128, OC2=64, OC3=32,
                 n_cores=8):
        assert C % 128 == 0
        self.C, self.H, self.W, self.K = C, H, W, K
        self.OC1, self.OC2, self.OC3 = OC1, OC2, OC3
        self.NCHUNK = C // 128
        self.n_cores = n_cores
        self.R = H // n_cores            # own rows per core
        self.Wp = W + 2                  # padded row width
        self.RX = self.R + 8             # x/fsm buffer rows (3 halo + 1 pad/side)
        self.PXB = self.RX * self.Wp
        self.RC1 = self.R + 6            # conv1 out rows (R+4) + 2 pad rows
        self.PC1 = self.RC1 * self.Wp
        self.P3 = self.R * self.Wp       # conv3/cov pixel region
        self.NB = -(-self.P3 // 128)     # 128-px blocks for S/cov
        self.NW3 = -(-self.P3 // 512)    # conv3 windows
        self.NQ3 = -(-self.NW3 // 4)     # conv3 quad groups (col-tiling x4)
        # conv2 A/B split (col-tiling x2): group A serves conv3 windows
        # [0, WS3), group B serves [WS3, NW3).
        self.WS3 = (self.NW3 + 1) // 2
        rA_max = -(-512 * self.WS3 // self.Wp)           # A needs rows -1..rA_max
        self.NA2 = rA_max + 2                            # rows -1 .. rA_max
        self.RB0 = (512 * self.WS3) // self.Wp           # first B conv3 out row
        self.NB2 = self.R - self.RB0 + 2                 # rows RB0-1 .. R
        self.PC2 = max(self.NA2, self.NB2) * self.Wp
        self.FCN = self.OC3 * self.OC3   # 1024
        self.FCCH = -(-self.FCN // 128)  # fc contraction chunks (8)
        assert 4 * self.Wp + 128 * self.NB <= self.PXB
        assert 128 * self.NB <= self.NW3 * 512
        # conv2 A reads conv1 rows up to (NA2-2)+1; B up to R+1  -> in range
        assert self.NA2 - 2 + 1 <= self.R + 2
        assert self.RB0 >= 1


def build(cfg: Cfg):
    nc = bacc.Bacc("TRN2", target_bir_lowering=False, debug=False,
                   num_devices=cfg.n_cores)
    C, K, Wp, R = cfg.C, cfg.K, cfg.Wp, cfg.R
    NCH = cfg.NCHUNK
    OC1, OC2, OC3 = cfg.OC1, cfg.OC2, cfg.OC3
    TAPS = [(dy, dx) for dy in (-1, 0, 1) for dx in (-1, 0, 1)]

    # ---------------- I/O ----------------
    x_in = nc.dram_tensor("x_slab", [NCH, 128, cfg.PXB], F32, kind="ExternalInput")
    sel_in = nc.dram_tensor("sel", [K, cfg.PXB], BF16, kind="ExternalInput")
    fgT_in = nc.dram_tensor("fgT", [128, cfg.NB, K], BF16, kind="ExternalInput")
    rc9_in = nc.dram_tensor("rc9", [K, 1], F32, kind="ExternalInput")
    w1_in = nc.dram_tensor("w1T", [9, NCH, 128, OC1], BF16, kind="ExternalInput")
    w2_in = nc.dram_tensor("w2T", [9, OC1, OC2], BF16, kind="ExternalInput")
    w3_in = nc.dram_tensor("w3T", [9, OC2, OC3], BF16, kind="ExternalInput")
    b1_in = nc.dram_tensor("b1c", [OC1, 1], F32, kind="ExternalInput")
    b2_in = nc.dram_tensor("b2c", [128, 1], F32, kind="ExternalInput")
    b3_in = nc.dram_tensor("b3c", [4 * OC3, 1], F32, kind="ExternalInput")
    fcw_in = nc.dram_tensor("fcwT", [cfg.FCCH, 128, cfg.FCN], F32R,
                            kind="ExternalInput")
    id128_in = nc.dram_tensor("iden128", [128, 128], F32, kind="ExternalInput")
    id32_in = nc.dram_tensor("iden32", [128, OC3], F32, kind="ExternalInput")
    mc1_in = nc.dram_tensor("mc1", [4, 1], F32, kind="ExternalInput")
    mc2_in = nc.dram_tensor("mc2", [2, 1], F32, kind="ExternalInput")

    fsm_out = nc.dram_tensor("fsm_o", [NCH, 128, R, cfg.W], F32,
                             kind="ExternalOutput")
    tr_out = nc.dram_tensor("tr_o", [K, cfg.FCN], F32, kind="ExternalOutput")

    with tile.TileContext(nc) as tc:
        with (
            tc.tile_pool(name="big", bufs=1) as big,
            tc.tile_pool(name="wts", bufs=1) as wts,
            tc.tile_pool(name="small", bufs=1) as small,
            tc.tile_pool(name="selp", bufs=2) as selp,
            tc.tile_pool(name="xt", bufs=3) as xtp,
            tc.tile_pool(name="fvt", bufs=2) as fvtp,
            tc.tile_pool(name="mst", bufs=2) as mstp,
            tc.tile_pool(name="fcw", bufs=2) as fcwp,
            tc.tile_pool(name="ps", bufs=4, space="PSUM") as psp,
            tc.tile_pool(name="dram", bufs=1, space="DRAM") as dram,
        ):
            # ------------- small/static loads first -------------
            id128 = small.tile([128, 128], F32, name="id128")
            nc.sync.dma_start(id128[:], id128_in.ap())
            id32 = small.tile([128, OC3], F32, name="id32")
            nc.sync.dma_start(id32[:], id32_in.ap())
            rc9 = small.tile([K, 1], F32, name="rc9")
            nc.sync.dma_start(rc9[:], rc9_in.ap())
            fgTs = wts.tile([128, cfg.NB, K], BF16, name="fgTs")
            nc.sync.dma_start(fgTs[:], fgT_in.ap())
            b1s = small.tile([OC1, 1], F32, name="b1s")
            nc.gpsimd.dma_start(b1s[:], b1_in.ap())
            b2s = small.tile([128, 1], F32, name="b2s")
            nc.gpsimd.dma_start(b2s[:], b2_in.ap())
            b3s = small.tile([4 * OC3, 1], F32, name="b3s")
            nc.gpsimd.dma_start(b3s[:], b3_in.ap())
            mc1s = small.tile([128, 4], F32, name="mc1s")
            _a = mc1_in.ap()
            nc.gpsimd.dma_start(mc1s[:], bass.AP(
                tensor=_a.tensor, offset=0, ap=[[0, 128]] + list(_a.ap)))
            mc2s = small.tile([128, 2], F32, name="mc2s")
            _a = mc2_in.ap()
            nc.gpsimd.dma_start(mc2s[:], bass.AP(
                tensor=_a.tensor, offset=0, ap=[[0, 128]] + list(_a.ap)))

            # ------------- big loads (x in row pieces) -------------
            xb = [big.tile([128, cfg.PXB], F32, name=f"xb{q}")
                  for q in range(NCH)]
            nrow4 = -(-cfg.RX // 4)
            for p in range(4):
                lo = p * nrow4 * Wp
                hi = min(cfg.PXB, (p + 1) * nrow4 * Wp)
                for q in range(NCH):
                    nc.sync.dma_start(xb[q][:, lo:hi], x_in.ap()[q][:, lo:hi])

            fsmbf = [big.tile([128, cfg.PXB], BF16, name=f"fsmbf{q}")
                     for q in range(NCH)]
            for q in range(NCH):
                nc.vector.memset(fsmbf[q][:, :Wp], 0.0)
                nc.vector.memset(fsmbf[q][:, (cfg.RX - 1) * Wp:], 0.0)

            c1b = big.tile([128, cfg.PC1 + 8], BF16, name="c1b")
            c2b = big.tile([128, 1 + cfg.PC2 + 8], BF16, name="c2b")
            fvb = big.tile([128, cfg.NQ3 * 512], F32, name="fvb")
            nc.vector.memset(c1b[:, :Wp], 0.0)
            nc.vector.memset(c1b[:, (cfg.RC1 - 1) * Wp:], 0.0)
            nc.vector.memset(c2b[:], 0.0)
            nc.vector.memset(fvb[:], 0.0)

            # conv weights (gpsimd queue; scattered descriptors)
            w1s = wts.tile([128, 9 * NCH, OC1], BF16, name="w1s")
            nc.gpsimd.dma_start(w1s[:], w1_in.ap().rearrange("t q i o -> i (t q) o"))
            w2s = wts.tile([OC1, 9, OC2], BF16, name="w2s")
            nc.gpsimd.dma_start(w2s[:], w2_in.ap().rearrange("t i o -> i t o"))
            w3s = wts.tile([128, 9, OC3], BF16, name="w3s")
            nc.gpsimd.dma_start(w3s[0:OC2], w3_in.ap().rearrange("t i o -> i t o"))
            nc.gpsimd.dma_start(w3s[OC2:2 * OC2],
                                w3_in.ap().rearrange("t i o -> i t o"))

            # ------------- phase A: masked channel sums S -------------
            # S[i, c] = sum_px fg[px, i] * x[c, px]  (raw 0/1 fg)
            s_acc = psp.tile([K, C], F32, name="s_acc", tag="acc", bufs=2)
            BB = 512 // (128 * NCH)          # S-blocks per PSUM bank
            for jj in range(0, cfg.NB, BB):
                blks = range(jj, min(jj + BB, cfg.NB))
                xt_ps = psp.tile([128, 512], F32, name="xt_ps", tag="tps",
                                 bufs=2)
                tidx = 0
                ntr = len(blks) * NCH
                tr_insts = []
                for j in blks:
                    off = 4 * Wp + 128 * j
                    for q in range(NCH):
                        tr_insts.append(nc.tensor.matmul(
                            xt_ps[:, 128 * (NCH * (j - jj) + q):
                                  128 * (NCH * (j - jj) + q + 1)],
                            xb[q][:, off:off + 128], id128[:],
                            is_transpose=True,
                            start=(tidx == 0), stop=(tidx == ntr - 1)))
                        tidx += 1
                _chain(tr_insts)
                for j in blks:
                    xt_sb = xtp.tile([128, 128 * NCH], BF16, name="xt_sb")
                    nc.vector.tensor_copy(
                        xt_sb[:],
                        xt_ps[:, 128 * NCH * (j - jj):128 * NCH * (j - jj + 1)])
                    nc.tensor.matmul(
                        s_acc[:], fgTs[:, j, :], xt_sb[:],
                        start=(j == 0), stop=(j == cfg.NB - 1))

            s_sb = small.tile([K, C], F32, name="s_sb")
            nc.vector.tensor_copy(s_sb[:], s_acc[:])

            ar_in = dram.tile([K, C], F32, name="ar_in")
            ar_out = dram.tile([K, C], F32, name="ar_out", addr_space="Shared")
            nc.gpsimd.dma_start(ar_in[:], s_sb[:])
            nc.gpsimd.collective_compute(
                "AllReduce", ALU.add,
                replica_groups=[list(range(cfg.n_cores))],
                ins=[ar_in.opt()], outs=[ar_out.opt()])
            means = small.tile([K, C], F32, name="means")
            nc.gpsimd.dma_start(means[:], ar_out[:])
            nc.vector.tensor_scalar_mul(means[:], means[:], rc9[:, 0:1])
            means_r = small.tile([K, C], BF16, name="means_r")
            nc.scalar.copy(means_r[:], means[:])

            # ------------- phase B: fsm = x - mean[sel] (in place) -------
            fs_lo, fs_hi = Wp, (cfg.RX - 1) * Wp
            w = fs_lo
            while w < fs_hi:
                nw = min(512, fs_hi - w)
                selw = selp.tile([K, 512], BF16, name="selw")
                nc.sync.dma_start(selw[:, :nw], sel_in.ap()[:, w:w + nw])
                for q in range(NCH):
                    msel = psp.tile([128, 512], F32, name="msel", tag="cps")
                    nc.tensor.matmul(
                        msel[:, :nw], means_r[:, 128 * q:128 * (q + 1)],
                        selw[:, :nw], start=True, stop=True)
                    nc.vector.tensor_tensor(
                        xb[q][:, w:w + nw], xb[q][:, w:w + nw],
                        msel[:, :nw], ALU.subtract)
                    nc.scalar.copy(fsmbf[q][:, w:w + nw], xb[q][:, w:w + nw])
                w += nw

            for q in range(NCH):
                nc.sync.dma_start(
                    fsm_out.ap()[q],
                    xb[q][:, :].rearrange(
                        "p (r u) -> p r u", r=cfg.RX)[:, 4:4 + R, 1:1 + cfg.W])

            # ------------- phase C: conv1 (C -> OC1, relu) -------------
            c1_lo, c1_hi = Wp, (cfg.RC1 - 1) * Wp
            wins = []
            w = c1_lo
            while w < c1_hi:
                wins.append((w, min(512, c1_hi - w)))
                w += 512
            for g in range(0, len(wins), 4):
                grp = wins[g:g + 4]
                psl = [psp.tile([128, 512], F32, name="c1ps", tag="cps")
                       for _ in grp]
                for ti, (dy, dx) in enumerate(TAPS):
                    for q in range(NCH):
                        tq = ti * NCH + q
                        for (s, nw), ps in zip(grp, psl):
                            nc.tensor.matmul(
                                ps[:OC1, :nw], w1s[:, tq, :],
                                fsmbf[q][:, s + Wp + dy * Wp + dx:
                                         s + Wp + dy * Wp + dx + nw],
                                start=(tq == 0), stop=(tq == 9 * NCH - 1))
                for (s, nw), ps in zip(grp, psl):
                    nc.scalar.activation(c1b[:OC1, s:s + nw], ps[:OC1, :nw],
                                         AF.Relu, bias=b1s[:, 0:1])
            c1v = c1b[:, :cfg.PC1].rearrange("p (r u) -> p r u", r=cfg.RC1)
            nc.vector.memset(c1v[:, 1:cfg.RC1 - 1, 0:1], 0.0)
            nc.vector.memset(c1v[:, 1:cfg.RC1 - 1, Wp - 1:Wp], 0.0)
            # zero conv1 rows outside the image (conv2 expects zero padding)
            c1m = c1b[:, Wp:3 * Wp].rearrange("p (a b) -> p a b", a=2)
            nc.vector.tensor_tensor(
                c1m, c1m, mc1s[:, 0:2].unsqueeze(2).broadcast_to(
                    [128, 2, Wp]), ALU.mult)
            c1m = c1b[:, (cfg.RC1 - 3) * Wp:(cfg.RC1 - 1) * Wp].rearrange(
                "p (a b) -> p a b", a=2)
            nc.vector.tensor_tensor(
                c1m, c1m, mc1s[:, 2:4].unsqueeze(2).broadcast_to(
                    [128, 2, Wp]), ALU.mult)

            # ------------- phase D: conv2 (OC1 -> OC2, relu), x2 tiled -----
            # A (psum/c2b partitions 0:64): image rows -1..NA2-2, buffer row
            # bA = r2 + 1.  B (partitions 64:128): rows RB0-1..R, buffer row
            # bB = r2 - RB0 + 1.  conv1 buffer row of image row r2 is r2 + 3.
            nwA = -(-cfg.NA2 * Wp // 512)
            nwB = -(-cfg.NB2 * Wp // 512)
            for g in range(max(nwA, nwB)):
                ps = psp.tile([128, 512], F32, name="c2ps", tag="cps")
                sA = 512 * g
                nA = min(512, cfg.NA2 * Wp - sA)
                sB = 512 * g
                nB = min(512, cfg.NB2 * Wp - sB)
                for ti, (dy, dx) in enumerate(TAPS):
                    if nA > 0:
                        nc.tensor.matmul(
                            ps[0:OC2, :nA], w2s[:, ti, :],
                            c1b[:, sA + (2 + dy) * Wp + dx:
                                sA + (2 + dy) * Wp + dx + nA],
                            start=(ti == 0), stop=(ti == 8),
                            tile_position=(0, 0), skip_group_check=True)
                    if nB > 0:
                        nc.tensor.matmul(
                            ps[64:64 + OC2, :nB], w2s[:, ti, :],
                            c1b[:, sB + (cfg.RB0 + 2 + dy) * Wp + dx:
                                sB + (cfg.RB0 + 2 + dy) * Wp + dx + nB],
                            start=(ti == 0), stop=(ti == 8),
                            tile_position=(0, 64), skip_group_check=True)
                if nA > 0:
                    nc.scalar.activation(
                        c2b[0:OC2, 1 + sA:1 + sA + nA], ps[0:OC2, :nA],
                        AF.Relu, bias=b2s[0:OC2, 0:1])
                if nB > 0:
                    nc.scalar.activation(
                        c2b[64:64 + OC2, 1 + sB:1 + sB + nB],
                        ps[64:64 + OC2, :nB],
                        AF.Relu, bias=b2s[64:128, 0:1])
            # re-zero pad columns (both groups share the column grid)
            c2v = c2b[:, 1:1 + cfg.PC2].rearrange("p (r u) -> p r u",
                                                  r=cfg.PC2 // Wp)
            nc.vector.memset(c2v[:, :, 0:1], 0.0)
            nc.vector.memset(c2v[:, :, Wp - 1:Wp], 0.0)
            # zero conv2 rows outside the image: A row 0 = image r0-1;
            # B row NB2-1 = image r1.
            nc.vector.tensor_tensor(
                c2b[0:OC2, 1:1 + Wp], c2b[0:OC2, 1:1 + Wp],
                mc2s[0:OC2, 0:1].broadcast_to([OC2, Wp]), ALU.mult)
            nc.vector.tensor_tensor(
                c2b[64:128, 1 + (cfg.NB2 - 1) * Wp:1 + cfg.NB2 * Wp],
                c2b[64:128, 1 + (cfg.NB2 - 1) * Wp:1 + cfg.NB2 * Wp],
                mc2s[64:128, 1:2].broadcast_to([OC2, Wp]), ALU.mult)

            # ------------- phase E: conv3 (OC2 -> OC3, +bias), x4 tiled ----
            for qd in range(cfg.NQ3):
                wlist = [wi for wi in range(4 * qd, min(4 * qd + 4, cfg.NW3))]
                ps = psp.tile([128, 512], F32, name="c3ps", tag="cps")
                for ti, (dy, dx) in enumerate(TAPS):
                    for wi in wlist:
                        s = 512 * wi
                        nw = min(512, cfg.P3 - s)
                        b = wi % 4
                        if wi < cfg.WS3:
                            off = 1 + s + (1 + dy) * Wp + dx
                            rhs = c2b[0:OC2, off:off + nw]
                        else:
                            off = 1 + s + (1 + dy - cfg.RB0) * Wp + dx
                            rhs = c2b[64:64 + OC2, off:off + nw]
                        wrow = 0 if wi < cfg.WS3 else OC2
                        nc.tensor.matmul(
                            ps[32 * b:32 * b + OC3, :nw],
                            w3s[wrow:wrow + OC2, ti, :], rhs,
                            start=(ti == 0), stop=(ti == 8),
                            tile_position=(wrow, 32 * b),
                            skip_group_check=True)
                for wi in wlist:
                    nw = min(512, cfg.P3 - 512 * wi)
                    b = wi % 4
                    nc.scalar.activation(
                        fvb[32 * b:32 * b + OC3, 512 * qd:512 * qd + nw],
                        ps[32 * b:32 * b + OC3, :nw],
                        AF.Identity, bias=b3s[32 * b:32 * b + OC3, 0:1])

            # ------------- phase F: per-mask covariance (f32r) -------------
            cov_ps = psp.tile([OC3, K * OC3], F32, name="cov_ps", tag="acc",
                              bufs=2)
            for wi in range(cfg.NW3):
                blks = [j for j in range(4 * wi, min(4 * wi + 4, cfg.NB))]
                if not blks:
                    break
                b = wi % 4
                k4 = wi // 4
                fvt_ps = psp.tile([128, 4 * OC3], F32, name="fvt_ps",
                                  tag="tps", bufs=2)
                tr_insts = []
                for j in blks:
                    woff = 128 * (j - 4 * wi)
                    tr_insts.append(nc.tensor.matmul(
                        fvt_ps[:, OC3 * (j - 4 * wi):OC3 * (j - 4 * wi + 1)],
                        fvb[32 * b:32 * b + OC3, 512 * k4 + woff:
                            512 * k4 + woff + 128],
                        id32[32 * b:32 * b + OC3, :],
                        is_transpose=True,
                        start=(j == blks[0]), stop=(j == blks[-1]),
                        tile_position=(32 * b, 0)))
                _chain(tr_insts)
                nb = OC3 * len(blks)
                fvt = fvtp.tile([128, 4 * OC3], F32R, name="fvt")
                nc.vector.tensor_copy(fvt[:, :nb], fvt_ps[:, :nb])
                for j in blks:
                    jo = OC3 * (j - 4 * wi)
                    mst = mstp.tile([128, K, OC3], F32R, name="mst")
                    nc.vector.tensor_tensor(
                        mst[:],
                        fvt[:, jo:jo + OC3].unsqueeze(1).broadcast_to(
                            [128, K, OC3]),
                        fgTs[:, j, :].unsqueeze(2).broadcast_to([128, K, OC3]),
                        ALU.mult)
                    nc.tensor.matmul(
                        cov_ps[:], fvt[:, jo:jo + OC3],
                        mst[:, :, :].rearrange("p k d -> p (k d)"),
                        start=(j == 0), stop=(j == cfg.NB - 1))

            cov_sb = small.tile([OC3, K * OC3], F32R, name="cov_sb")
            nc.vector.tensor_copy(cov_sb[:], cov_ps[:])

            # rearrange cov[c,(k,d)] -> covT[(c4,d), (ci,k)] via DRAM bounce
            ckcd = dram.tile([K, OC3, OC3], F32R, name="ckcd")
            nc.gpsimd.dma_start(
                ckcd.rearrange("k c d -> c k d"),
                cov_sb.rearrange("c (k d) -> c k d", k=K))
            covTs = small.tile([128, cfg.FCCH, K], F32R, name="covTs")
            for ci in range(cfg.FCCH):
                nc.gpsimd.dma_start(
                    covTs[:, ci, :],
                    ckcd[:, 4 * ci:4 * ci + 4, :].rearrange(
                        "k c d -> (c d) k"))

            # ------------- phase G: fc (partial trans, f32r) -------------
            NJ = cfg.FCN
            halves = [(h, min(512, NJ - h)) for h in range(0, NJ, 512)]
            tr_ps = [psp.tile([K, min(512, NJ)], F32, name=f"tr_ps{i}",
                              tag="acc", bufs=2) for i in range(len(halves))]
            for ci in range(cfg.FCCH):
                for (h, nh), ps in zip(halves, tr_ps):
                    fcw = fcwp.tile([128, 512], F32R, name="fcw")
                    nc.sync.dma_start(fcw[:, :nh], fcw_in.ap()[ci, :, h:h + nh])
                    nc.tensor.matmul(
                        ps[:, :nh], covTs[:, ci, :], fcw[:, :nh],
                        start=(ci == 0), stop=(ci == cfg.FCCH - 1))
            tr_sb = small.tile([K, NJ], F32, name="tr_sb")
            for (h, nh), ps in zip(halves, tr_ps):
                nc.vector.tensor_copy(tr_sb[:, h:h + nh], ps[:, :nh])
            nc.sync.dma_start(tr_out.ap()[:], tr_sb[:])

    nc.compile()
    return nc


# ============================ host side ============================

def prep_inputs(cfg: Cfg, x, masks, w1, b1, w2, b2, w3, b3, fc_w, fc_b):
    C, H, W, K, Wp, R = cfg.C, cfg.H, cfg.W, cfg.K, cfg.Wp, cfg.R
    xv = np.asarray(x, np.float32).reshape(C, H, W)
    m = np.asarray(masks)
    fg = (m > 0).astype(np.float32)                      # [K, H, W]
    counts = fg.reshape(K, -1).sum(1)
    valid = counts >= 10.0
    recip = (1.0 / np.maximum(counts, 1.0)).astype(np.float32)

    # one-hot of the last valid mask covering each pixel
    sel = np.zeros((K, H, W), np.float32)
    covered = np.zeros((H, W), bool)
    for i in range(K - 1, -1, -1):
        if not valid[i]:
            continue
        on = fg[i] > 0
        sel[i][on & ~covered] = 1.0
        covered |= on

    w1 = np.asarray(w1, np.float32)
    w2 = np.asarray(w2, np.float32)
    w3 = np.asarray(w3, np.float32)
    w1T = np.ascontiguousarray(np.transpose(
        w1.reshape(cfg.OC1, cfg.NCHUNK, 128, 3, 3),
        (3, 4, 1, 2, 0)).reshape(9, cfg.NCHUNK, 128, cfg.OC1)).astype(BFNP)
    w2T = np.ascontiguousarray(np.transpose(w2, (2, 3, 1, 0)).reshape(
        9, cfg.OC1, cfg.OC2)).astype(BFNP)
    w3T = np.ascontiguousarray(np.transpose(w3, (2, 3, 1, 0)).reshape(
        9, cfg.OC2, cfg.OC3)).astype(BFNP)
    fcwT = np.ascontiguousarray(
        np.asarray(fc_w, np.float32).T.reshape(cfg.FCCH, 128, cfg.FCN))
    id128 = np.eye(128, dtype=np.float32)
    id32 = np.tile(np.eye(cfg.OC3, dtype=np.float32), (128 // cfg.OC3, 1))
    b1c = np.asarray(b1, np.float32).reshape(-1, 1)
    b2c = np.tile(np.asarray(b2, np.float32), 2).reshape(-1, 1)
    b3c = np.tile(np.asarray(b3, np.float32), 4).reshape(-1, 1)

    in_maps = []
    for c in range(cfg.n_cores):
        r0 = c * R
        xs = np.zeros((cfg.NCHUNK, 128, cfg.RX, Wp), np.float32)
        ss = np.zeros((K, cfg.RX, Wp), np.float32)
        for b in range(1, cfg.RX - 1):
            r = r0 - 4 + b
            if 0 <= r < H:
                xs[:, :, b, 1:1 + W] = xv.reshape(cfg.NCHUNK, 128, H, W)[:, :, r, :]
                ss[:, b, 1:1 + W] = sel[:, r, :]
        fgw = np.zeros((cfg.RX, Wp, K), np.float32)
        fgw[4:4 + R, 1:1 + W, :] = np.transpose(fg[:, r0:r0 + R, :], (1, 2, 0))
        fgflat = fgw.reshape(-1, K)
        base = 4 * Wp
        fgT = np.transpose(
            fgflat[base:base + cfg.NB * 128].reshape(cfg.NB, 128, K),
            (1, 0, 2)).astype(BFNP)
        mc1 = np.array([[1.0 if 0 <= r < H else 0.0]
                        for r in (r0 - 2, r0 - 1, r0 + R, r0 + R + 1)],
                       np.float32)
        mc2 = np.array([[1.0 if 0 <= r < H else 0.0]
                        for r in (r0 - 1, r0 + R)], np.float32)
        in_maps.append({
            "mc1": mc1, "mc2": mc2,
            "x_slab": xs.reshape(cfg.NCHUNK, 128, cfg.PXB),
            "sel": ss.reshape(K, cfg.PXB).astype(BFNP),
            "fgT": np.ascontiguousarray(fgT),
            "rc9": recip.reshape(K, 1),
            "w1T": w1T, "w2T": w2T, "w3T": w3T,
            "b1c": b1c, "b2c": b2c, "b3c": b3c,
            "fcwT": fcwT, "iden128": id128, "iden32": id32,
        })
    return in_maps, valid, counts


def assemble(cfg: Cfg, results, valid, fc_b, recip):
    K = cfg.K
    fsm = np.zeros((cfg.C, cfg.H, cfg.W), np.float32)
    for c, res in enumerate(results):
        r0 = c * cfg.R
        fo = res["fsm_o"]           # [NCH, 128, R, W]
        for q in range(cfg.NCHUNK):
            fsm[q * 128:(q + 1) * 128, r0:r0 + cfg.R, :] = fo[q]
    trans = np.zeros((K, cfg.FCN), np.float32)
    for res in results:
        trans += res["tr_o"]
    trans *= recip[:, None]
    trans = trans + np.asarray(fc_b, np.float32)[None, :]
    trans[~valid] = 0.0
    return trans.astype(np.float32), fsm.reshape(cfg.C, cfg.H * cfg.W)


_CACHE = {}


def _get_nc(cfg: Cfg):
    key = (cfg.C, cfg.H, cfg.W, cfg.n_cores)
    if key not in _CACHE:
        _CACHE[key] = build(cfg)
    return _CACHE[key]


def kernel(x, masks, w1, b1, w2, b2, w3, b3, fc_w, fc_b, **run_kwargs):
    cfg = Cfg(C=x.shape[1], H=x.shape[2], W=x.shape[3])
    nc = _get_nc(cfg)
    in_maps, valid, counts = prep_inputs(cfg, x, masks, w1, b1, w2, b2, w3,
                                         b3, fc_w, fc_b)
    recip = (1.0 / np.maximum(counts, 1.0)).astype(np.float32)
    res = run_bass_kernel_spmd(nc, in_maps, core_ids=list(range(cfg.n_cores)),
                               **run_kwargs)
    out = assemble(cfg, res.results, valid, fc_b, recip)
    kernel.last_results = res
    return out


# revision 28
# speedup vs baseline: 1.2275x; 1.0766x over previous
"""Trainium2 Bass kernel for nn_CNN_90546500534707 (segment_reduce).

Pipeline (per the reference):
  1. per-mask channel means over masked pixels, sequential overwrite ->
     fsm = x - mean[last valid mask covering pixel]  (output 1)
  2. conv stack 256->128->64->32 (3x3, SAME, relu on first two)
  3. per-mask covariance of conv features (32x32) -> fc 1024->1024 (output 0)

Sharding: image rows split across 8 cores (32 rows + 3-row halo each).
Everything is laid out in a zero-padded row geometry (width W+2) so 3x3
convs become 9 shifted matmuls accumulated in PSUM.  The only cross-core
exchange is one AllReduce of the per-mask channel sums [9, 256]; the
per-mask covariance and fc are linear in the per-core partial covariance,
so each core emits a partial `trans` and the host sums/scales them.

Precision: x, fsm, means and the masked-sum path stay fp32/near-exact; the
conv stack runs bf16 (fp32 PSUM accumulation); covariance + fc run f32r.
"""

import sys

sys.path.insert(0, "/opt/trn_rl_repo")

import numpy as np
import ml_dtypes

import concourse.bass as bass
import concourse.tile as tile
from concourse import bacc, mybir
from concourse.bass_utils import run_bass_kernel_spmd
from concourse.tile_rust import add_dep_helper

F32 = mybir.dt.float32
F32R = mybir.dt.float32r
BF16 = mybir.dt.bfloat16
AF = mybir.ActivationFunctionType
ALU = mybir.AluOpType
BFNP = ml_dtypes.bfloat16


def _chain(insts):
    """Order matmuls of one PSUM bank group (start must execute first)."""
    for a, b in zip(insts[1:], insts):
        add_dep_helper(a.ins, b.ins, sync=False, reason="psum group order")


class Cfg:
    def __init__(self, C=256, H=256, W=256, K=9, OC1=128, OC2=64, OC3=32,
                 n_cores=8):
        assert C % 128 == 0
        self.C, self.H, self.W, self.K = C, H, W, K
        self.OC1, self.OC2, self.OC3 = OC1, OC2, OC3
        self.NCHUNK = C // 128
        self.n_cores = n_cores
        self.R = H // n_cores            # own rows per core
        self.Wp = W + 2                  # padded row width
        self.RX = self.R + 8             # x/fsm buffer rows (3 halo + 1 pad/side)
        self.PXB = self.RX * self.Wp
        self.RC1 = self.R + 6            # conv1 out rows (R+4) + 2 pad rows
        self.PC1 = self.RC1 * self.Wp
        self.P3 = self.R * self.Wp       # conv3/cov pixel region
        self.NB = -(-self.P3 // 128)     # 128-px blocks for S/cov
        self.NW3 = -(-self.P3 // 512)    # conv3 windows
        self.NQ3 = -(-self.NW3 // 4)     # conv3 quad groups (col-tiling x4)
        # conv2 A/B split (col-tiling x2): group A serves conv3 windows
        # [0, WS3), group B serves [WS3, NW3).
        self.WS3 = (self.NW3 + 1) // 2
        rA_max = -(-512 * self.WS3 // self.Wp)           # A needs rows -1..rA_max
        self.NA2 = rA_max + 2                            # rows -1 .. rA_max
        self.RB0 = (512 * self.WS3) // self.Wp           # first B conv3 out row
        self.NB2 = self.R - self.RB0 + 2                 # rows RB0-1 .. R
        self.PC2 = max(self.NA2, self.NB2) * self.Wp
        self.FCN = self.OC3 * self.OC3   # 1024
        self.FCCH = -(-self.FCN // 128)  # fc contraction chunks (8)
        assert 4 * self.Wp + 128 * self.NB <= self.PXB
        assert 128 * self.NB <= self.NW3 * 512
        # conv2 A reads conv1 rows up to (NA2-2)+1; B up to R+1  -> in range
        assert self.NA2 - 2 + 1 <= self.R + 2
        assert self.RB0 >= 1


def build(cfg: Cfg):
    nc = bacc.Bacc("TRN2", target_bir_lowering=False, debug=False,
                   num_devices=cfg.n_cores)
    C, K, Wp, R = cfg.C, cfg.K, cfg.Wp, cfg.R
    NCH = cfg.NCHUNK
    OC1, OC2, OC3 = cfg.OC1, cfg.OC2, cfg.OC3
    TAPS = [(dy, dx) for dy in (-1, 0, 1) for dx in (-1, 0, 1)]

    # ---------------- I/O ----------------
    x_in = nc.dram_tensor("x_slab", [NCH, 128, cfg.PXB], F32, kind="ExternalInput")
    sel_in = nc.dram_tensor("sel", [K, cfg.PXB], BF16, kind="ExternalInput")
    fgT_in = nc.dram_tensor("fgT", [128, cfg.NB, K], BF16, kind="ExternalInput")
    rc9_in = nc.dram_tensor("rc9", [K, 1], F32, kind="ExternalInput")
    w1_in = nc.dram_tensor("w1T", [9, NCH, 128, OC1], BF16, kind="ExternalInput")
    w2_in = nc.dram_tensor("w2T", [9, OC1, OC2], BF16, kind="ExternalInput")
    w3_in = nc.dram_tensor("w3T", [9, OC2, OC3], BF16, kind="ExternalInput")
    b1_in = nc.dram_tensor("b1c", [OC1, 1], F32, kind="ExternalInput")
    b2_in = nc.dram_tensor("b2c", [128, 1], F32, kind="ExternalInput")
    b3_in = nc.dram_tensor("b3c", [4 * OC3, 1], F32, kind="ExternalInput")
    fcw_in = nc.dram_tensor("fcwT", [cfg.FCCH, 128, cfg.FCN], F32R,
                            kind="ExternalInput")
    id128_in = nc.dram_tensor("iden128", [128, 128], F32, kind="ExternalInput")
    id32_in = nc.dram_tensor("iden32", [128, OC3], F32, kind="ExternalInput")
    mc1_in = nc.dram_tensor("mc1", [4, 1], F32, kind="ExternalInput")
    mc2_in = nc.dram_tensor("mc2", [2, 1], F32, kind="ExternalInput")

    fsm_out = nc.dram_tensor("fsm_o", [NCH, 128, R, cfg.W], F32,
                             kind="ExternalOutput")
    tr_out = nc.dram_tensor("tr_o", [K, cfg.FCN], F32, kind="ExternalOutput")

    with tile.TileContext(nc) as tc:
        with (
            tc.tile_pool(name="big", bufs=1) as big,
            tc.tile_pool(name="wts", bufs=1) as wts,
            tc.tile_pool(name="small", bufs=1) as small,
            tc.tile_pool(name="selp", bufs=2) as selp,
            tc.tile_pool(name="xt", bufs=3) as xtp,
            tc.tile_pool(name="fvt", bufs=2) as fvtp,
            tc.tile_pool(name="mst", bufs=2) as mstp,
            tc.tile_pool(name="fcw", bufs=2) as fcwp,
            tc.tile_pool(name="ps", bufs=4, space="PSUM") as psp,
            tc.tile_pool(name="dram", bufs=1, space="DRAM") as dram,
        ):
            # ------------- small/static loads first -------------
            id128 = small.tile([128, 128], F32, name="id128")
            nc.sync.dma_start(id128[:], id128_in.ap())
            id32 = small.tile([128, OC3], F32, name="id32")
            nc.sync.dma_start(id32[:], id32_in.ap())
            rc9 = small.tile([K, 1], F32, name="rc9")
            nc.sync.dma_start(rc9[:], rc9_in.ap())
            fgTs = wts.tile([128, cfg.NB, K], BF16, name="fgTs")
            nc.sync.dma_start(fgTs[:], fgT_in.ap())
            b1s = small.tile([OC1, 1], F32, name="b1s")
            nc.gpsimd.dma_start(b1s[:], b1_in.ap())
            b2s = small.tile([128, 1], F32, name="b2s")
            nc.gpsimd.dma_start(b2s[:], b2_in.ap())
            b3s = small.tile([4 * OC3, 1], F32, name="b3s")
            nc.gpsimd.dma_start(b3s[:], b3_in.ap())
            mc1s = small.tile([128, 4], F32, name="mc1s")
            _a = mc1_in.ap()
            nc.gpsimd.dma_start(mc1s[:], bass.AP(
                tensor=_a.tensor, offset=0, ap=[[0, 128]] + list(_a.ap)))
            mc2s = small.tile([128, 2], F32, name="mc2s")
            _a = mc2_in.ap()
            nc.gpsimd.dma_start(mc2s[:], bass.AP(
                tensor=_a.tensor, offset=0, ap=[[0, 128]] + list(_a.ap)))

            # ------------- big loads (x in row pieces) -------------
            xb = [big.tile([128, cfg.PXB], F32, name=f"xb{q}")
                  for q in range(NCH)]
            nrow4 = -(-cfg.RX // 4)
            for p in range(4):
                lo = p * nrow4 * Wp
                hi = min(cfg.PXB, (p + 1) * nrow4 * Wp)
                for q in range(NCH):
                    nc.sync.dma_start(xb[q][:, lo:hi], x_in.ap()[q][:, lo:hi])

            fsmbf = [big.tile([128, cfg.PXB], BF16, name=f"fsmbf{q}")
                     for q in range(NCH)]
            for q in range(NCH):
                nc.vector.memset(fsmbf[q][:, :Wp], 0.0)
                nc.vector.memset(fsmbf[q][:, (cfg.RX - 1) * Wp:], 0.0)

            c1b = big.tile([128, cfg.PC1 + 8], BF16, name="c1b")
            c2b = big.tile([128, 1 + cfg.PC2 + 8], BF16, name="c2b")
            fvb = big.tile([128, cfg.NQ3 * 512], F32, name="fvb")
            nc.vector.memset(c1b[:, :Wp], 0.0)
            nc.vector.memset(c1b[:, (cfg.RC1 - 1) * Wp:], 0.0)
            nc.vector.memset(c2b[:], 0.0)
            nc.vector.memset(fvb[:], 0.0)

            # conv weights (gpsimd queue; scattered descriptors)
            w1s = wts.tile([128, 9 * NCH, OC1], BF16, name="w1s")
            nc.gpsimd.dma_start(w1s[:], w1_in.ap().rearrange("t q i o -> i (t q) o"))
            w2s = wts.tile([OC1, 9, OC2], BF16, name="w2s")
            nc.gpsimd.dma_start(w2s[:], w2_in.ap().rearrange("t i o -> i t o"))
            w3s = wts.tile([128, 9, OC3], BF16, name="w3s")
            nc.gpsimd.dma_start(w3s[0:OC2], w3_in.ap().rearrange("t i o -> i t o"))
            nc.gpsimd.dma_start(w3s[OC2:2 * OC2],
                                w3_in.ap().rearrange("t i o -> i t o"))

            # ------------- phase A: masked channel sums S -------------
            # S[i, c] = sum_px fg[px, i] * x[c, px]  (raw 0/1 fg)
            s_acc = psp.tile([K, C], F32, name="s_acc", tag="acc", bufs=2)
            BB = 512 // (128 * NCH)          # S-blocks per PSUM bank
            for jj in range(0, cfg.NB, BB):
                blks = range(jj, min(jj + BB, cfg.NB))
                xt_ps = psp.tile([128, 512], F32, name="xt_ps", tag="tps",
                                 bufs=2)
                tidx = 0
                ntr = len(blks) * NCH
                tr_insts = []
                for j in blks:
                    off = 4 * Wp + 128 * j
                    for q in range(NCH):
                        tr_insts.append(nc.tensor.matmul(
                            xt_ps[:, 128 * (NCH * (j - jj) + q):
                                  128 * (NCH * (j - jj) + q + 1)],
                            xb[q][:, off:off + 128], id128[:],
                            is_transpose=True,
                            start=(tidx == 0), stop=(tidx == ntr - 1)))
                        tidx += 1
                _chain(tr_insts)
                nbl = 128 * NCH * len(blks)
                xt_sb = xtp.tile([128, 512], BF16, name="xt_sb")
                nc.vector.tensor_copy(xt_sb[:, :nbl], xt_ps[:, :nbl])
                for j in blks:
                    nc.tensor.matmul(
                        s_acc[:], fgTs[:, j, :],
                        xt_sb[:, 128 * NCH * (j - jj):128 * NCH * (j - jj + 1)],
                        start=(j == 0), stop=(j == cfg.NB - 1))

            s_sb = small.tile([K, C], F32, name="s_sb")
            nc.vector.tensor_copy(s_sb[:], s_acc[:])

            ar_in = dram.tile([K, C], F32, name="ar_in")
            ar_out = dram.tile([K, C], F32, name="ar_out", addr_space="Shared")
            nc.gpsimd.dma_start(ar_in[:], s_sb[:])
            nc.gpsimd.collective_compute(
                "AllReduce", ALU.add,
                replica_groups=[list(range(cfg.n_cores))],
                ins=[ar_in.opt()], outs=[ar_out.opt()])
            means = small.tile([K, C], F32, name="means")
            nc.gpsimd.dma_start(means[:], ar_out[:])
            nc.vector.tensor_scalar_mul(means[:], means[:], rc9[:, 0:1])
            means_r = small.tile([K, C], BF16, name="means_r")
            nc.scalar.copy(means_r[:], means[:])

            # ------------- phase B: fsm = x - mean[sel] (in place) -------
            fs_lo, fs_hi = Wp, (cfg.RX - 1) * Wp
            w = fs_lo
            while w < fs_hi:
                nw = min(512, fs_hi - w)
                selw = selp.tile([K, 512], BF16, name="selw")
                nc.sync.dma_start(selw[:, :nw], sel_in.ap()[:, w:w + nw])
                for q in range(NCH):
                    msel = psp.tile([128, 512], F32, name="msel", tag="cps")
                    nc.tensor.matmul(
                        msel[:, :nw], means_r[:, 128 * q:128 * (q + 1)],
                        selw[:, :nw], start=True, stop=True)
                    nc.vector.tensor_tensor(
                        xb[q][:, w:w + nw], xb[q][:, w:w + nw],
                        msel[:, :nw], ALU.subtract)
                    nc.scalar.copy(fsmbf[q][:, w:w + nw], xb[q][:, w:w + nw])
                w += nw

            for q in range(NCH):
                nc.sync.dma_start(
                    fsm_out.ap()[q],
                    xb[q][:, :].rearrange(
                        "p (r u) -> p r u", r=cfg.RX)[:, 4:4 + R, 1:1 + cfg.W])

            # ------------- phase C: conv1 (C -> OC1, relu) -------------
            c1_lo, c1_hi = Wp, (cfg.RC1 - 1) * Wp
            wins = []
            w = c1_lo
            while w < c1_hi:
                wins.append((w, min(512, c1_hi - w)))
                w += 512
            for g in range(0, len(wins), 4):
                grp = wins[g:g + 4]
                psl = [psp.tile([128, 512], F32, name="c1ps", tag="cps")
                       for _ in grp]
                for ti, (dy, dx) in enumerate(TAPS):
                    for q in range(NCH):
                        tq = ti * NCH + q
                        for (s, nw), ps in zip(grp, psl):
                            nc.tensor.matmul(
                                ps[:OC1, :nw], w1s[:, tq, :],
                                fsmbf[q][:, s + Wp + dy * Wp + dx:
                                         s + Wp + dy * Wp + dx + nw],
                                start=(tq == 0), stop=(tq == 9 * NCH - 1))
                for (s, nw), ps in zip(grp, psl):
                    nc.scalar.activation(c1b[:OC1, s:s + nw], ps[:OC1, :nw],
                                         AF.Relu, bias=b1s[:, 0:1])
            c1v = c1b[:, :cfg.PC1].rearrange("p (r u) -> p r u", r=cfg.RC1)
            nc.vector.memset(c1v[:, 1:cfg.RC1 - 1, 0:1], 0.0)
            nc.vector.memset(c1v[:, 1:cfg.RC1 - 1, Wp - 1:Wp], 0.0)
            # zero conv1 rows outside the image (conv2 expects zero padding)
            c1m = c1b[:, Wp:3 * Wp].rearrange("p (a b) -> p a b", a=2)
            nc.vector.tensor_tensor(
                c1m, c1m, mc1s[:, 0:2].unsqueeze(2).broadcast_to(
                    [128, 2, Wp]), ALU.mult)
            c1m = c1b[:, (cfg.RC1 - 3) * Wp:(cfg.RC1 - 1) * Wp].rearrange(
                "p (a b) -> p a b", a=2)
            nc.vector.tensor_tensor(
                c1m, c1m, mc1s[:, 2:4].unsqueeze(2).broadcast_to(
                    [128, 2, Wp]), ALU.mult)

            # ------------- phase D: conv2 (OC1 -> OC2, relu), x2 tiled -----
            # A (psum/c2b partitions 0:64): image rows -1..NA2-2, buffer row
            # bA = r2 + 1.  B (partitions 64:128): rows RB0-1..R, buffer row
            # bB = r2 - RB0 + 1.  conv1 buffer row of image row r2 is r2 + 3.
            nwA = -(-cfg.NA2 * Wp // 512)
            nwB = -(-cfg.NB2 * Wp // 512)
            for g in range(max(nwA, nwB)):
                ps = psp.tile([128, 512], F32, name="c2ps", tag="cps")
                sA = 512 * g
                nA = min(512, cfg.NA2 * Wp - sA)
                sB = 512 * g
                nB = min(512, cfg.NB2 * Wp - sB)
                for ti, (dy, dx) in enumerate(TAPS):
                    if nA > 0:
                        nc.tensor.matmul(
                            ps[0:OC2, :nA], w2s[:, ti, :],
                            c1b[:, sA + (2 + dy) * Wp + dx:
                                sA + (2 + dy) * Wp + dx + nA],
                            start=(ti == 0), stop=(ti == 8),
                            tile_position=(0, 0), skip_group_check=True)
                    if nB > 0:
                        nc.tensor.matmul(
                            ps[64:64 + OC2, :nB], w2s[:, ti, :],
                            c1b[:, sB + (cfg.RB0 + 2 + dy) * Wp + dx:
                                sB + (cfg.RB0 + 2 + dy) * Wp + dx + nB],
                            start=(ti == 0), stop=(ti == 8),
                            tile_position=(0, 64), skip_group_check=True)
                if nA > 0:
                    nc.scalar.activation(
                        c2b[0:OC2, 1 + sA:1 + sA + nA], ps[0:OC2, :nA],
                        AF.Relu, bias=b2s[0:OC2, 0:1])
                if nB > 0:
                    nc.scalar.activation(
                        c2b[64:64 + OC2, 1 + sB:1 + sB + nB],
                        ps[64:64 + OC2, :nB],
                        AF.Relu, bias=b2s[64:128, 0:1])
            # re-zero pad columns (both groups share the column grid)
            c2v = c2b[:, 1:1 + cfg.PC2].rearrange("p (r u) -> p r u",
                                                  r=cfg.PC2 // Wp)
            nc.vector.memset(c2v[:, :, 0:1], 0.0)
            nc.vector.memset(c2v[:, :, Wp - 1:Wp], 0.0)
            # zero conv2 rows outside the image: A row 0 = image r0-1;
            # B row NB2-1 = image r1.
            nc.vector.tensor_tensor(
                c2b[0:OC2, 1:1 + Wp], c2b[0:OC2, 1:1 + Wp],
                mc2s[0:OC2, 0:1].broadcast_to([OC2, Wp]), ALU.mult)
            nc.vector.tensor_tensor(
                c2b[64:128, 1 + (cfg.NB2 - 1) * Wp:1 + cfg.NB2 * Wp],
                c2b[64:128, 1 + (cfg.NB2 - 1) * Wp:1 + cfg.NB2 * Wp],
                mc2s[64:128, 1:2].broadcast_to([OC2, Wp]), ALU.mult)

            # ------------- phase E: conv3 (OC2 -> OC3, +bias), x4 tiled ----
            for qd in range(cfg.NQ3):
                wlist = [wi for wi in range(4 * qd, min(4 * qd + 4, cfg.NW3))]
                ps = psp.tile([128, 512], F32, name="c3ps", tag="cps")
                for ti, (dy, dx) in enumerate(TAPS):
                    for wi in wlist:
                        s = 512 * wi
                        nw = min(512, cfg.P3 - s)
                        b = wi % 4
                        if wi < cfg.WS3:
                            off = 1 + s + (1 + dy) * Wp + dx
                            rhs = c2b[0:OC2, off:off + nw]
                        else:
                            off = 1 + s + (1 + dy - cfg.RB0) * Wp + dx
                            rhs = c2b[64:64 + OC2, off:off + nw]
                        wrow = 0 if wi < cfg.WS3 else OC2
                        nc.tensor.matmul(
                            ps[32 * b:32 * b + OC3, :nw],
                            w3s[wrow:wrow + OC2, ti, :], rhs,
                            start=(ti == 0), stop=(ti == 8),
                            tile_position=(wrow, 32 * b),
                            skip_group_check=True)
                for wi in wlist:
                    nw = min(512, cfg.P3 - 512 * wi)
                    b = wi % 4
                    nc.scalar.activation(
                        fvb[32 * b:32 * b + OC3, 512 * qd:512 * qd + nw],
                        ps[32 * b:32 * b + OC3, :nw],
                        AF.Identity, bias=b3s[32 * b:32 * b + OC3, 0:1])

            # ------------- phase F: per-mask covariance (f32r) -------------
            cov_ps = psp.tile([OC3, K * OC3], F32, name="cov_ps", tag="acc",
                              bufs=2)
            for wi in range(cfg.NW3):
                blks = [j for j in range(4 * wi, min(4 * wi + 4, cfg.NB))]
                if not blks:
                    break
                b = wi % 4
                k4 = wi // 4
                fvt_ps = psp.tile([128, 4 * OC3], F32, name="fvt_ps",
                                  tag="tps", bufs=2)
                tr_insts = []
                for j in blks:
                    woff = 128 * (j - 4 * wi)
                    tr_insts.append(nc.tensor.matmul(
                        fvt_ps[:, OC3 * (j - 4 * wi):OC3 * (j - 4 * wi + 1)],
                        fvb[32 * b:32 * b + OC3, 512 * k4 + woff:
                            512 * k4 + woff + 128],
                        id32[32 * b:32 * b + OC3, :],
                        is_transpose=True,
                        start=(j == blks[0]), stop=(j == blks[-1]),
                        tile_position=(32 * b, 0)))
                _chain(tr_insts)
                nb = OC3 * len(blks)
                fvt = fvtp.tile([128, 4 * OC3], F32R, name="fvt")
                nc.vector.tensor_copy(fvt[:, :nb], fvt_ps[:, :nb])
                for j in blks:
                    jo = OC3 * (j - 4 * wi)
                    mst = mstp.tile([128, K, OC3], F32R, name="mst")
                    nc.vector.tensor_tensor(
                        mst[:],
                        fvt[:, jo:jo + OC3].unsqueeze(1).broadcast_to(
                            [128, K, OC3]),
                        fgTs[:, j, :].unsqueeze(2).broadcast_to([128, K, OC3]),
                        ALU.mult)
                    nc.tensor.matmul(
                        cov_ps[:], fvt[:, jo:jo + OC3],
                        mst[:, :, :].rearrange("p k d -> p (k d)"),
                        start=(j == 0), stop=(j == cfg.NB - 1))

            cov_sb = small.tile([OC3, K * OC3], F32R, name="cov_sb")
            nc.vector.tensor_copy(cov_sb[:], cov_ps[:])

            # rearrange cov[c,(k,d)] -> covT[(c4,d), (ci,k)] via DRAM bounce
            ckcd = dram.tile([K, OC3, OC3], F32R, name="ckcd")
            nc.gpsimd.dma_start(
                ckcd.rearrange("k c d -> c k d"),
                cov_sb.rearrange("c (k d) -> c k d", k=K))
            covTs = small.tile([128, cfg.FCCH, K], F32R, name="covTs")
            for ci in range(cfg.FCCH):
                nc.gpsimd.dma_start(
                    covTs[:, ci, :],
                    ckcd[:, 4 * ci:4 * ci + 4, :].rearrange(
                        "k c d -> (c d) k"))

            # ------------- phase G: fc (partial trans, f32r) -------------
            NJ = cfg.FCN
            halves = [(h, min(512, NJ - h)) for h in range(0, NJ, 512)]
            tr_ps = [psp.tile([K, min(512, NJ)], F32, name=f"tr_ps{i}",
                              tag="acc", bufs=2) for i in range(len(halves))]
            for ci in range(cfg.FCCH):
                for (h, nh), ps in zip(halves, tr_ps):
                    fcw = fcwp.tile([128, 512], F32R, name="fcw")
                    nc.sync.dma_start(fcw[:, :nh], fcw_in.ap()[ci, :, h:h + nh])
                    nc.tensor.matmul(
                        ps[:, :nh], covTs[:, ci, :], fcw[:, :nh],
                        start=(ci == 0), stop=(ci == cfg.FCCH - 1))
            tr_sb = small.tile([K, NJ], F32, name="tr_sb")
            for (h, nh), ps in zip(halves, tr_ps):
                nc.vector.tensor_copy(tr_sb[:, h:h + nh], ps[:, :nh])
            nc.sync.dma_start(tr_out.ap()[:], tr_sb[:])

    nc.compile()
    return nc


# ============================ host side ============================

def prep_inputs(cfg: Cfg, x, masks, w1, b1, w2, b2, w3, b3, fc_w, fc_b):
    C, H, W, K, Wp, R = cfg.C, cfg.H, cfg.W, cfg.K, cfg.Wp, cfg.R
    xv = np.asarray(x, np.float32).reshape(C, H, W)
    m = np.asarray(masks)
    fg = (m > 0).astype(np.float32)                      # [K, H, W]
    counts = fg.reshape(K, -1).sum(1)
    valid = counts >= 10.0
    recip = (1.0 / np.maximum(counts, 1.0)).astype(np.float32)

    # one-hot of the last valid mask covering each pixel
    sel = np.zeros((K, H, W), np.float32)
    covered = np.zeros((H, W), bool)
    for i in range(K - 1, -1, -1):
        if not valid[i]:
            continue
        on = fg[i] > 0
        sel[i][on & ~covered] = 1.0
        covered |= on

    w1 = np.asarray(w1, np.float32)
    w2 = np.asarray(w2, np.float32)
    w3 = np.asarray(w3, np.float32)
    w1T = np.ascontiguousarray(np.transpose(
        w1.reshape(cfg.OC1, cfg.NCHUNK, 128, 3, 3),
        (3, 4, 1, 2, 0)).reshape(9, cfg.NCHUNK, 128, cfg.OC1)).astype(BFNP)
    w2T = np.ascontiguousarray(np.transpose(w2, (2, 3, 1, 0)).reshape(
        9, cfg.OC1, cfg.OC2)).astype(BFNP)
    w3T = np.ascontiguousarray(np.transpose(w3, (2, 3, 1, 0)).reshape(
        9, cfg.OC2, cfg.OC3)).astype(BFNP)
    fcwT = np.ascontiguousarray(
        np.asarray(fc_w, np.float32).T.reshape(cfg.FCCH, 128, cfg.FCN))
    id128 = np.eye(128, dtype=np.float32)
    id32 = np.tile(np.eye(cfg.OC3, dtype=np.float32), (128 // cfg.OC3, 1))
    b1c = np.asarray(b1, np.float32).reshape(-1, 1)
    b2c = np.tile(np.asarray(b2, np.float32), 2).reshape(-1, 1)
    b3c = np.tile(np.asarray(b3, np.float32), 4).reshape(-1, 1)

    in_maps = []
    for c in range(cfg.n_cores):
        r0 = c * R
        xs = np.zeros((cfg.NCHUNK, 128, cfg.RX, Wp), np.float32)
        ss = np.zeros((K, cfg.RX, Wp), np.float32)
        for b in range(1, cfg.RX - 1):
            r = r0 - 4 + b
            if 0 <= r < H:
                xs[:, :, b, 1:1 + W] = xv.reshape(cfg.NCHUNK, 128, H, W)[:, :, r, :]
                ss[:, b, 1:1 + W] = sel[:, r, :]
        fgw = np.zeros((cfg.RX, Wp, K), np.float32)
        fgw[4:4 + R, 1:1 + W, :] = np.transpose(fg[:, r0:r0 + R, :], (1, 2, 0))
        fgflat = fgw.reshape(-1, K)
        base = 4 * Wp
        fgT = np.transpose(
            fgflat[base:base + cfg.NB * 128].reshape(cfg.NB, 128, K),
            (1, 0, 2)).astype(BFNP)
        mc1 = np.array([[1.0 if 0 <= r < H else 0.0]
                        for r in (r0 - 2, r0 - 1, r0 + R, r0 + R + 1)],
                       np.float32)
        mc2 = np.array([[1.0 if 0 <= r < H else 0.0]
                        for r in (r0 - 1, r0 + R)], np.float32)
        in_maps.append({
            "mc1": mc1, "mc2": mc2,
            "x_slab": xs.reshape(cfg.NCHUNK, 128, cfg.PXB),
            "sel": ss.reshape(K, cfg.PXB).astype(BFNP),
            "fgT": np.ascontiguousarray(fgT),
            "rc9": recip.reshape(K, 1),
            "w1T": w1T, "w2T": w2T, "w3T": w3T,
            "b1c": b1c, "b2c": b2c, "b3c": b3c,
            "fcwT": fcwT, "iden128": id128, "iden32": id32,
        })
    return in_maps, valid, counts


def assemble(cfg: Cfg, results, valid, fc_b, recip):
    K = cfg.K
    fsm = np.zeros((cfg.C, cfg.H, cfg.W), np.float32)
    for c, res in enumerate(results):
        r0 = c * cfg.R
        fo = res["fsm_o"]           # [NCH, 128, R, W]
        for q in range(cfg.NCHUNK):
            fsm[q * 128:(q + 1) * 128, r0:r0 + cfg.R, :] = fo[q]
    trans = np.zeros((K, cfg.FCN), np.float32)
    for res in results:
        trans += res["tr_o"]
    trans *= recip[:, None]
    trans = trans + np.asarray(fc_b, np.float32)[None, :]
    trans[~valid] = 0.0
    return trans.astype(np.float32), fsm.reshape(cfg.C, cfg.H * cfg.W)


_CACHE = {}


def _get_nc(cfg: Cfg):
    key = (cfg.C, cfg.H, cfg.W, cfg.n_cores)
    if key not in _CACHE:
        _CACHE[key] = build(cfg)
    return _CACHE[key]


def kernel(x, masks, w1, b1, w2, b2, w3, b3, fc_w, fc_b, **run_kwargs):
    cfg = Cfg(C=x.shape[1], H=x.shape[2], W=x.shape[3])
    nc = _get_nc(cfg)
    in_maps, valid, counts = prep_inputs(cfg, x, masks, w1, b1, w2, b2, w3,
                                         b3, fc_w, fc_b)
    recip = (1.0 / np.maximum(counts, 1.0)).astype(np.float32)
    res = run_bass_kernel_spmd(nc, in_maps, core_ids=list(range(cfg.n_cores)),
                               **run_kwargs)
    out = assemble(cfg, res.results, valid, fc_b, recip)
    kernel.last_results = res
    return out
